# revision 7
# baseline (speedup 1.0000x reference)
"""Trainium2 Bass kernel for nn_Net_47596827574272 (gnn_message_passing).

Pipeline: pf_encode MLP -> 3x DynamicEdgeConv (shared weights) -> global_add_pool
-> output MLP.  The three edge convs (kNN graph rebuild + edge MLP + max
aggregation) run on 8 NeuronCores, data-parallel over the 256 graphs (32 graphs
per core, whole graphs per core so kNN/gather/max stay local).  The pooling and
the tiny output MLP run on host.

Key algebra used on device (T-layout: features on partitions, nodes on free):
  edge-MLP pre-activation z[n,m] = u[n] + v[m] with
    u = h @ (Wc1 - Wc2) (+ fused bias), v = h @ Wc2
  so max-aggregation over the K=24 nearest neighbours is
    out[n] = elu(u[n] + max_{m in knn(n)} v[m])  (elu is monotone).
  elu(x) + 1 = max(x + 1, exp(x)) exactly, so the "+1"-shifted activations are
  computed in 2 ops; the shift is folded into the next layer's bias (distances
  are shift-invariant) and subtracted on host at the end.
  Top-24 per node via 3 rounds of vector.max/max_index/match_replace; neighbour
  gather as a one-hot float32r matmul; max over K via strided tensor_reduce.
"""
import sys
import numpy as np

sys.path.insert(0, "/opt/trn_rl_repo")

import concourse.bacc as bacc
import concourse.mybir as mybir
import concourse.tile as tile
from concourse.bass_utils import run_bass_kernel_spmd

dt = mybir.dt
Alu = mybir.AluOpType
Act = mybir.ActivationFunctionType

NCORES = 8
B = 256            # graphs
N = 64             # nodes per graph
K = 24             # neighbours
HID = 128
GPC = B // NCORES  # graphs per core
NPC = GPC * N      # nodes per core = 2048
NSTACK = GPC // 2  # 16 two-graph stacks per core
CH = 512           # matmul moving-dim chunk


def _build_nc(dbg=False):
    nc = bacc.Bacc("TRN2")

    # ---- I/O ----
    xT = nc.dram_tensor("xT", [16, NPC], dt.float32, kind="ExternalInput")
    W1a = nc.dram_tensor("W1a", [16, HID], dt.float32, kind="ExternalInput")
    W2 = nc.dram_tensor("W2", [HID, HID], dt.float32, kind="ExternalInput")
    b2p = nc.dram_tensor("b2p", [HID, 1], dt.float32, kind="ExternalInput")
    b2p1 = nc.dram_tensor("b2p1", [HID, 1], dt.float32, kind="ExternalInput")
    Wa = nc.dram_tensor("Wa", [HID, HID], dt.float32, kind="ExternalInput")
    Wv = nc.dram_tensor("Wv", [HID, HID], dt.float32, kind="ExternalInput")
    bcp = nc.dram_tensor("bcp", [HID, 1], dt.float32, kind="ExternalInput")
    bcp1 = nc.dram_tensor("bcp1", [HID, 1], dt.float32, kind="ExternalInput")
    ident = nc.dram_tensor("ident", [128, 128], dt.float32r, kind="ExternalInput")
    ones1 = nc.dram_tensor("ones1", [1, 128], dt.float32r, kind="ExternalInput")
    ones128 = nc.dram_tensor("ones128", [128, 128], dt.float32, kind="ExternalInput")
    iota_neg = nc.dram_tensor("iota_neg", [128, 1], dt.float32, kind="ExternalInput")
    off64 = nc.dram_tensor("off64", [128, 1], dt.float32, kind="ExternalInput")
    f3p = nc.dram_tensor("f3p", [HID, NPC], dt.float32, kind="ExternalOutput")
    if dbg:
        dbg_h = nc.dram_tensor("dbg_h", [HID, NPC], dt.float32, kind="ExternalOutput")
        dbg_f = [nc.dram_tensor(f"dbg_f{t}", [HID, NPC], dt.float32, kind="ExternalOutput")
                 for t in range(3)]
        dbg_idx = [nc.dram_tensor(f"dbg_idx{t}", [NPC // 128, 128, K], dt.float32, kind="ExternalOutput")
                   for t in range(3)]
        dbg_maxv = [nc.dram_tensor(f"dbg_maxv{t}", [NPC // 128, 128, 128], dt.float32, kind="ExternalOutput")
                    for t in range(3)]

    with tile.TileContext(nc) as tc:
        with (
            tc.tile_pool(name="consts", bufs=1) as cpool,
            tc.tile_pool(name="big", bufs=1) as bpool,
            tc.tile_pool(name="work", bufs=2) as wpool,
            tc.tile_pool(name="ps512", bufs=2, space="PSUM") as ps512,
            tc.tile_pool(name="psG", bufs=1, space="PSUM") as psG,
            tc.tile_pool(name="psgram", bufs=1, space="PSUM") as psgram,
            tc.tile_pool(name="ps128", bufs=1, space="PSUM") as ps128,
        ):
            xT_s = cpool.tile_from(xT[:, :])
            W1a_s = cpool.tile_from(W1a[:, :])
            W2_s = cpool.tile_from(W2[:, :])
            b2p_s = cpool.tile_from(b2p[:, :])
            b2p1_s = cpool.tile_from(b2p1[:, :])
            Wa_s = cpool.tile_from(Wa[:, :])
            Wv_s = cpool.tile_from(Wv[:, :])
            bcp_s = cpool.tile_from(bcp[:, :])
            bcp1_s = cpool.tile_from(bcp1[:, :])
            ident_s = cpool.tile_from(ident[:, :])
            ones1_s = cpool.tile_from(ones1[:, :])
            ones128_s = cpool.tile_from(ones128[:, :])
            iota_neg_s = cpool.tile_from(iota_neg[:, :])
            off64_s = cpool.tile_from(off64[:, :])

            hpA = bpool.tile([HID, NPC], dt.float32)
            hpB = bpool.tile([HID, NPC], dt.float32)
            sqb = bpool.tile([128, NPC], dt.float32)
            usb = bpool.tile([HID, NPC], dt.float32)

            # ---------- encoder ----------
            # elu(z)+1 == relu(z) + exp(min(z, 0)), computed exactly.
            # layer 1: z1 = W1a.T @ [x;1] (bias folded via the ones row)
            for c in range(0, NPC, CH):
                pz = ps512.tile([128, CH], dt.float32, tag="ps512")
                nc.tensor.matmul(pz[:, :], W1a_s[:, :], xT_s[:, c:c + CH],
                                 start=True, stop=True)
                m = wpool.tile([128, CH], dt.float32, tag="mtile")
                nc.vector.tensor_scalar(m[:, :], pz[:, :], 0.0, None, op0=Alu.min)
                e = wpool.tile([128, CH], dt.float32, tag="etile")
                nc.scalar.activation(e[:, :], m[:, :], Act.Exp)
                r = wpool.tile([128, CH], dt.float32, tag="rtile")
                nc.scalar.activation(r[:, :], pz[:, :], Act.Relu)
                nc.vector.tensor_add(hpB[:, c:c + CH], r[:, :], e[:, :])
            # layer 2: z2 = W2.T @ h1' + b2p
            for c in range(0, NPC, CH):
                pz = ps512.tile([128, CH], dt.float32, tag="ps512")
                nc.tensor.matmul(pz[:, :], W2_s[:, :], hpB[:, c:c + CH],
                                 start=True, stop=True)
                m = wpool.tile([128, CH], dt.float32, tag="mtile")
                nc.vector.tensor_scalar(m[:, :], pz[:, :], b2p_s[:, 0:1], 0.0,
                                        op0=Alu.add, op1=Alu.min)
                e = wpool.tile([128, CH], dt.float32, tag="etile")
                nc.scalar.activation(e[:, :], m[:, :], Act.Exp)
                r = wpool.tile([128, CH], dt.float32, tag="rtile")
                nc.scalar.activation(r[:, :], pz[:, :], Act.Relu,
                                     bias=b2p_s[:, 0:1])
                nc.vector.tensor_add(hpA[:, c:c + CH], r[:, :], e[:, :])

            # ---------- 3 edge convs ----------
            if dbg:
                nc.sync.dma_start(dbg_h[:, :], hpA[:, :])
            hp_in, hp_out = hpA, hpB
            for conv in range(3):
                # squared norms broadcast: sqb[:, j] = sum_c hp[c, j]^2 (all rows)
                for c in range(0, NPC, CH):
                    h2 = wpool.tile([128, CH], dt.float32, tag="h2tile")
                    nc.scalar.activation(h2[:, :], hp_in[:, c:c + CH], Act.Square)
                    pq = ps512.tile([128, CH], dt.float32, tag="ps512")
                    nc.tensor.matmul(pq[:, :], ones128_s[:, :], h2[:, :],
                                     start=True, stop=True)
                    nc.scalar.copy(sqb[:, c:c + CH], pq[:, :])
                # u = Wa.T @ hp  (bias folded into the final activation)
                for c in range(0, NPC, CH):
                    pu = ps512.tile([128, CH], dt.float32, tag="ps512")
                    nc.tensor.matmul(pu[:, :], Wa_s[:, :], hp_in[:, c:c + CH],
                                     start=True, stop=True)
                    nc.scalar.copy(usb[:, c:c + CH], pu[:, :])

                for s in range(NSTACK):
                    sl = slice(128 * s, 128 * s + 128)
                    g0 = slice(128 * s, 128 * s + 64)
                    g1 = slice(128 * s + 64, 128 * s + 128)

                    # Gram (within graph), packed two graphs per [128, 64] tile
                    pg = psgram.tile([128, 64], dt.float32, tag="gram")
                    nc.tensor.matmul(pg[0:64, :], hp_in[:, g0], hp_in[:, g0],
                                     start=True, stop=True)
                    nc.tensor.matmul(pg[64:128, :], hp_in[:, g1], hp_in[:, g1],
                                     start=True, stop=True)
                    # score = 2*G - sq[m]
                    score = wpool.tile([128, 64], dt.float32, tag="score")
                    nc.vector.scalar_tensor_tensor(score[0:64, :], pg[0:64, :], 2.0,
                                                   sqb[0:64, g0], op0=Alu.mult,
                                                   op1=Alu.subtract)
                    nc.vector.scalar_tensor_tensor(score[64:128, :], pg[64:128, :], 2.0,
                                                   sqb[64:128, g1], op0=Alu.mult,
                                                   op1=Alu.subtract)
                    # top-24 (3 rounds of max8 + index + zap)
                    m8 = wpool.tile([128, K], dt.float32, tag="m8")
                    i8 = wpool.tile([128, K], dt.uint32, tag="i8")
                    for r in range(3):
                        r8 = slice(8 * r, 8 * r + 8)
                        nc.vector.max(m8[:, r8], score[:, :])
                        nc.vector.max_index(i8[:, r8], m8[:, r8], score[:, :])
                        nc.vector.match_replace(score[:, :], m8[:, r8],
                                                score[:, :], -3e38)
                    # stack-global fp index
                    idxf = wpool.tile([128, K], dt.float32r, tag="idxf")
                    nc.vector.tensor_scalar(idxf[:, :], i8[:, :], off64_s[:, 0:1],
                                            None, op0=Alu.add)
                    if dbg:
                        nc.sync.dma_start(dbg_idx[conv][s, :, :],
                                          idxf[:, :].bitcast(dt.float32))
                    # transpose [128,24] -> [24,128], linearize k-major
                    pt = ps128.tile([K, 128], dt.float32r, tag="ps128")
                    nc.tensor.transpose(pt[:, :], idxf[:, :], ident_s[:, :])
                    idxT_sb = wpool.tile([K, 128], dt.float32r, tag="idxT_sb")
                    nc.scalar.copy(idxT_sb[:, :], pt[:, :])
                    idxflat = wpool.tile([1, K * 128], dt.float32r, tag="idxflat")
                    nc.sync.dma_start(idxflat[0:1, :], idxT_sb[:, :])

                    # v values for this stack: [m2, h]
                    pv = ps128.tile([128, 128], dt.float32, tag="ps128v")
                    nc.tensor.matmul(pv[:, :], hp_in[:, sl], Wv_s[:, :],
                                     start=True, stop=True)
                    vsb = wpool.tile([128, 128], dt.float32r, tag="vsb")
                    nc.scalar.copy(vsb[:, :], pv[:, :])

                    # one-hot [m2, (k, n)] = (idx broadcast == partition index)
                    a1 = wpool.tile([128, K * 128], dt.float32, tag="a1")
                    onehot = wpool.tile([128, K * 128], dt.float32r, tag="onehot")
                    for ci, c in enumerate(range(0, K * 128, CH)):
                        pb = ps512.tile([128, CH], dt.float32, tag="ps512")
                        nc.tensor.matmul(pb[:, :], ones1_s[:, :],
                                         idxflat[:, c:c + CH], start=True, stop=True)
                        nc.scalar.activation(a1[:, c:c + CH], pb[:, :], Act.Abs,
                                             bias=iota_neg_s[:, 0:1])
                        if ci < 3:
                            nc.scalar.activation(onehot[:, c:c + CH], a1[:, c:c + CH],
                                                 Act.Relu, bias=1.0, scale=-1.0)
                        else:
                            nc.gpsimd.tensor_scalar(onehot[:, c:c + CH],
                                                    a1[:, c:c + CH], 0.5, None,
                                                    op0=Alu.is_le)
                    # gather + max over k, in two k-halves
                    maxv = wpool.tile([128, 128], dt.float32, tag="maxv")
                    for half in range(2):
                        pG = psG.tile([128, (K // 2) * 128], dt.float32, tag="G")
                        base = half * (K // 2) * 128
                        for c in range(0, (K // 2) * 128, CH):
                            nc.tensor.matmul(pG[:, c:c + CH], vsb[:, :],
                                             onehot[:, base + c:base + c + CH],
                                             start=True, stop=True)
                        mh = wpool.tile([128, 128], dt.float32, tag=f"maxvh{half}")
                        nc.vector.tensor_reduce(
                            mh[:, :],
                            pG[:, :].rearrange("h (k n) -> h n k", k=K // 2),
                            axis=mybir.AxisListType.X, op=Alu.max)
                        if half == 0:
                            mh0 = mh
                        else:
                            nc.vector.tensor_max(maxv[:, :], mh0[:, :], mh[:, :])
                    if dbg:
                        nc.sync.dma_start(dbg_maxv[conv][s, :, :], maxv[:, :])
                    # combine: f' = relu(zb) + exp(min(zb, 0)), zb = u + maxv + bcp
                    s2 = wpool.tile([128, 128], dt.float32, tag="s2")
                    nc.vector.tensor_add(s2[:, :], usb[:, sl], maxv[:, :])
                    m2 = wpool.tile([128, 128], dt.float32, tag="m2")
                    nc.vector.tensor_scalar(m2[:, :], s2[:, :], bcp_s[:, 0:1], 0.0,
                                            op0=Alu.add, op1=Alu.min)
                    e2 = wpool.tile([128, 128], dt.float32, tag="e2")
                    nc.scalar.activation(e2[:, :], m2[:, :], Act.Exp)
                    r2 = wpool.tile([128, 128], dt.float32, tag="r2")
                    nc.scalar.activation(r2[:, :], s2[:, :], Act.Relu,
                                         bias=bcp_s[:, 0:1])
                    nc.vector.tensor_add(hp_out[:, sl], r2[:, :], e2[:, :])
                if dbg:
                    nc.sync.dma_start(dbg_f[conv][:, :], hp_out[:, :])
                hp_in, hp_out = hp_out, hp_in

            nc.sync.dma_start(f3p[:, :], hp_in[:, :])

    nc.compile()
    return nc


_NC = None


def _get_nc():
    global _NC
    if _NC is None:
        _NC = _build_nc()
    return _NC


def _prep_consts(W1, b1, W2, b2, Wc, bc):
    W1 = np.asarray(W1, np.float32)
    b1 = np.asarray(b1, np.float32)
    W2 = np.asarray(W2, np.float32)
    b2 = np.asarray(b2, np.float32)
    Wc = np.asarray(Wc, np.float32)
    bc = np.asarray(bc, np.float32)
    Wc1, Wc2 = Wc[:HID], Wc[HID:]
    W1a = np.concatenate([W1, b1[None, :]], axis=0)          # [16, 128]
    b2p = b2 - W2.sum(axis=0)
    Wa = Wc1 - Wc2
    bcp = bc - Wc1.sum(axis=0)
    consts = {
        "W1a": W1a,
        "W2": W2,
        "b2p": b2p.reshape(HID, 1),
        "b2p1": (b2p + 1.0).reshape(HID, 1),
        "Wa": Wa,
        "Wv": Wc2,
        "bcp": bcp.reshape(HID, 1),
        "bcp1": (bcp + 1.0).reshape(HID, 1),
        "ident": np.eye(128, dtype=np.float32),
        "ones1": np.ones((1, 128), np.float32),
        "ones128": np.ones((128, 128), np.float32),
        "iota_neg": -np.arange(128, dtype=np.float32).reshape(128, 1),
        "off64": np.concatenate([np.zeros(64, np.float32),
                                 np.full(64, 64.0, np.float32)]).reshape(128, 1),
    }
    return {k: np.ascontiguousarray(v, dtype=np.float32) for k, v in consts.items()}


def _elu(x):
    return np.where(x > 0, x, np.expm1(np.minimum(x, 0.0)))


def kernel(x_pf, batch_pf, W1, b1, W2, b2, Wc, bc,
           Wo1, bo1, Wo2, bo2, Wo3, bo3, Wo4, bo4):
    x_pf = np.asarray(x_pf, np.float32)
    consts = _prep_consts(W1, b1, W2, b2, Wc, bc)

    in_maps = []
    for i in range(NCORES):
        xs = x_pf[i * NPC:(i + 1) * NPC]                      # [2048, 15]
        xT = np.concatenate([xs.T, np.ones((1, NPC), np.float32)], axis=0)
        m = dict(consts)
        m["xT"] = np.ascontiguousarray(xT)
        in_maps.append(m)

    nc = _get_nc()
    res = run_bass_kernel_spmd(nc, in_maps, list(range(NCORES)))

    # f3 (true, unshifted), node-major
    f3 = np.concatenate([res.results[i]["f3p"].T for i in range(NCORES)],
                        axis=0).astype(np.float32) - 1.0      # [16384, 128]

    # global_add_pool by batch_pf, then output MLP (host, tiny)
    batch_np = np.asarray(batch_pf)
    pooled = np.zeros((B, HID), np.float32)
    np.add.at(pooled, batch_np.astype(np.int64), f3)

    o = _elu(pooled @ np.asarray(Wo1, np.float32) + np.asarray(bo1, np.float32))
    o = _elu(o @ np.asarray(Wo2, np.float32) + np.asarray(bo2, np.float32))
    o = _elu(o @ np.asarray(Wo3, np.float32) + np.asarray(bo3, np.float32))
    out = o @ np.asarray(Wo4, np.float32) + np.asarray(bo4, np.float32)
    return (out.astype(np.float32), batch_pf)
